# revision 9
# baseline (speedup 1.0000x reference)
"""FP64->FP32 bit-circuit converter for Trainium2 (8 NeuronCores), packed I/O.

The end-to-end cost of kernel() is dominated by host<->device transport over
the axon tunnel (~50-150 MB/s, high fixed latency), not device execution.
Strategy (pure data parallel over the batch, 131072 rows/core):

  host:   pack the (B, 64) {0,1}-float input into 2 int32 words per row
          (8 MB instead of 256 MB) with an XLA-CPU jit, in two halves so the
          first half's async upload overlaps packing of the second half (the
          two global arrays also double the number of transfer streams);
  device: run the whole conversion as ~35 fused int32 ALU ops per row on the
          vector engine (int32 bitwise is DVE-only; Pool-engine int ops
          measured ~30x slower), emitting the literal IEEE fp32 bit pattern
          as one int32 per row (4 MB back instead of 128 MB);
  host:   expand the words back into the (B, 32) float bit matrix.

Bit layout (MSB-first, matching the column order of the reference):
  hi = row bits 0..31  (bit0=sign at bit31, bits1..11=exp, bits12..31=mant0..19)
  lo = row bits 32..63 (mant bits 20..51)
Output word = sign<<31 | exp8<<23 | mant23 -- exactly the fp32 bit pattern,
whose MSB-first bit expansion equals the 32 output columns.

Half h of the input feeds supertile h: partition p of tensor x<h> holds core
rows [p*1024 + h*512, p*1024 + (h+1)*512), so the output row order matches.

The Bass kernel is compiled and first executed via
bass_utils.run_bass_kernel_spmd (during warm-up, which also cross-checks the
fast path against it); steady-state calls reuse one cached jit executor with
donated output buffers created on-device.  Warm-up starts in a background
thread at import so axon connection + neuronxcc compile overlap harness setup.
"""
import threading
import numpy as np
import jax
import jax.numpy as jnp
from jax.sharding import Mesh, PartitionSpec, NamedSharding
from jax.experimental.shard_map import shard_map

from concourse import bacc, bass2jax, mybir
from concourse.tile import TileContext
from concourse.bass_utils import run_bass_kernel_spmd

AOT = mybir.AluOpType
I32 = mybir.dt.int32

B = 1_048_576
N_CORES = 8
B_CORE = B // N_CORES          # 131072
P = 128                        # SBUF partitions
NI = B_CORE // P               # 1024 rows per partition
NF = NI // 2                   # 512 rows/partition per supertile
BH = B_CORE // 2               # 65536 rows per half per core
D_IN = 2                       # packed words per row


def _build():
    nc = bacc.Bacc("TRN2")
    x0 = nc.dram_tensor("x0", [BH, D_IN], I32, kind="ExternalInput")
    x1 = nc.dram_tensor("x1", [BH, D_IN], I32, kind="ExternalInput")
    y = nc.dram_tensor("y", [B_CORE, 1], I32, kind="ExternalOutput")
    y_r = y.ap().rearrange("(p n) d -> p (n d)", p=P)
    xrs = [t.ap().rearrange("(p n) d -> p (n d)", p=P) for t in (x0, x1)]

    with TileContext(nc) as tc:
        with (
            tc.tile_pool(name="io", bufs=2) as io,
            tc.tile_pool(name="sc", bufs=2) as sc,
        ):
            for st in range(2):
                off = st * NF
                xin = io.tile([P, NF * D_IN], I32, tag="xin", name="xin")
                nc.sync.dma_start(xin[:, :], xrs[st][:, :])
                xv = xin[:, :].rearrange("p (n d) -> p n d", d=D_IN)
                hi = xv[:, :, 0]
                lo = xv[:, :, 1]

                def T(tag):
                    t = sc.tile([P, NF], I32, tag=tag, name=tag)
                    return t[:, :]

                V = nc.vector
                # field extraction
                E = T("E")          # 11-bit biased fp64 exponent
                V.tensor_scalar(E, hi, 20, 0x7FF,
                                AOT.logical_shift_right, AOT.bitwise_and)
                Mhi = T("Mhi")
                V.tensor_scalar(Mhi, hi, 0xFFFFF, 3,
                                AOT.bitwise_and, AOT.logical_shift_left)
                Mlo = T("Mlo")
                V.tensor_scalar(Mlo, lo, 29, 7,
                                AOT.logical_shift_right, AOT.bitwise_and)
                M23 = T("M23")      # top 23 mantissa bits as an int
                V.tensor_tensor(M23, Mhi, Mlo, AOT.bitwise_or)
                R = T("R")          # round bit (mant bit 23)
                V.tensor_scalar(R, lo, 28, 1,
                                AOT.logical_shift_right, AOT.bitwise_and)
                sval = T("sval")    # sticky field (mant bits 24..51)
                V.tensor_scalar(sval, lo, 0x0FFFFFFF, None, AOT.bitwise_and)
                S = T("S")
                V.tensor_scalar(S, sval, 1, None, AOT.min)
                # round-to-nearest-even: round_up = R & (S | lsb)
                L = T("L")
                V.tensor_scalar(L, M23, 1, None, AOT.bitwise_and)
                SL = T("SL")
                V.tensor_tensor(SL, S, L, AOT.bitwise_or)
                ru = T("ru")
                V.tensor_tensor(ru, R, SL, AOT.bitwise_and)
                Mr = T("Mr")
                V.tensor_tensor(Mr, M23, ru, AOT.add)
                c_m = T("c_m")      # mantissa carry into the exponent
                V.tensor_scalar(c_m, Mr, 23, None, AOT.logical_shift_right)
                mant_f = T("mant_f")
                V.tensor_scalar(mant_f, Mr, 0x7FFFFF, None, AOT.bitwise_and)
                # rebias: newE = (E - 896) + c_m
                newE = T("newE")
                V.scalar_tensor_tensor(newE, E, -896, c_m, AOT.add, AOT.add)
                nsh = T("nsh")
                V.tensor_scalar(nsh, newE, 23, None, AOT.logical_shift_left)
                body = T("body")
                V.tensor_tensor(body, nsh, mant_f, AOT.bitwise_or)
                # specials
                over = T("over")
                V.tensor_scalar(over, E, 1151, None, AOT.is_ge)
                under = T("under")
                V.tensor_scalar(under, E, 897, None, AOT.is_lt)
                lml = T("lml")      # mant bits 23..51
                V.tensor_scalar(lml, lo, 0x1FFFFFFF, None, AOT.bitwise_and)
                manyv = T("manyv")
                V.tensor_tensor(manyv, M23, lml, AOT.bitwise_or)
                eq2047 = T("eq2047")
                V.tensor_scalar(eq2047, E, 2047, None, AOT.is_equal)
                many = T("many")
                V.tensor_scalar(many, manyv, 1, None, AOT.min)
                nan = T("nan")
                V.tensor_tensor(nan, eq2047, many, AOT.bitwise_and)
                om = T("om")
                V.tensor_scalar(om, over, 1, None, AOT.subtract)
                um = T("um")
                V.tensor_scalar(um, under, 1, None, AOT.subtract)
                nm = T("nm")
                V.tensor_scalar(nm, nan, 1, None, AOT.subtract)
                sb = T("sb")
                V.tensor_scalar(sb, hi, 31, 31,
                                AOT.logical_shift_right, AOT.logical_shift_left)
                # body1 = over ? 0x7F800000 : body   (xor/and with NOT-mask)
                x1t = T("x1t")
                V.tensor_scalar(x1t, body, 0x7F800000, None, AOT.bitwise_xor)
                x2t = T("x2t")
                V.tensor_tensor(x2t, x1t, om, AOT.bitwise_and)
                body1 = T("body1")
                V.tensor_scalar(body1, x2t, 0x7F800000, None, AOT.bitwise_xor)
                # body2 = under ? 0 : body1
                body2 = T("body2")
                V.tensor_tensor(body2, body1, um, AOT.bitwise_and)
                # body3 = nan ? 0x7FC00000 : body2
                x3t = T("x3t")
                V.tensor_scalar(x3t, body2, 0x7FC00000, None, AOT.bitwise_xor)
                x4t = T("x4t")
                V.tensor_tensor(x4t, x3t, nm, AOT.bitwise_and)
                body3 = T("body3")
                V.tensor_scalar(body3, x4t, 0x7FC00000, None, AOT.bitwise_xor)
                yt = io.tile([P, NF], I32, tag="yt", name="yt")
                V.tensor_tensor(yt[:, :], body3, sb, AOT.bitwise_or)
                nc.sync.dma_start(y_r[:, off:off + NF], yt[:, :])
    nc.compile()
    return nc


# ---------------- host-side pack / unpack (XLA CPU) ----------------
_PACK_W = (np.uint32(1) << np.arange(31, -1, -1, dtype=np.uint32)).astype(np.int32)


def _pack_half_cpu(xf, h):
    # partition p of half h holds core rows [p*NI + h*NF, p*NI + (h+1)*NF)
    xr = xf.reshape(N_CORES, P, 2, NF, 64)[:, :, h]
    xi = xr.reshape(-1, D_IN, 32).astype(jnp.int32)
    return (xi * _PACK_W[None, None, :]).sum(axis=-1, dtype=jnp.int32)


def _unpack_cpu(w):
    sh = jnp.arange(31, -1, -1, dtype=jnp.int32)
    bits = jnp.right_shift(w.reshape(-1, 1).view(jnp.uint32),
                           sh.view(jnp.uint32)[None, :]) & jnp.uint32(1)
    return bits.astype(jnp.float32)


def _pack_half_np(x: np.ndarray, h: int) -> np.ndarray:
    xr = np.ascontiguousarray(
        x.reshape(N_CORES, P, 2, NF, 64)[:, :, h]).reshape(-1, 64)
    xp = np.packbits(xr != 0, axis=-1)
    return xp.view(np.dtype(">u4")).astype(np.uint32).view(np.int32)


def _unpack_output_np(w: np.ndarray) -> np.ndarray:
    wbe = w.view(np.uint32).astype(np.dtype(">u4"))
    bits = np.unpackbits(wbe.view(np.uint8).reshape(-1, 4), axis=-1)
    return bits.astype(np.float32)


# ---------------- cached executor ----------------
_STATE: dict = {}
_LOCK = threading.Lock()


def _prepare_locked():
    if "ready" in _STATE or "failed" in _STATE:
        return
    try:
        nc = _build()
        _STATE["nc"] = nc

        # official path first: compile + run the Bass kernel via
        # run_bass_kernel_spmd (dummy input); also warms devices + NEFF.
        dummy = np.zeros((BH, D_IN), np.int32)
        in_maps = [{"x0": dummy, "x1": dummy} for _ in range(N_CORES)]
        res = run_bass_kernel_spmd(nc, in_maps, core_ids=list(range(N_CORES)))
        w_official = np.concatenate([r["y"] for r in res.results], axis=0)

        pack_jits = [jax.jit(lambda x, hh=h: _pack_half_cpu(x, hh),
                             backend="cpu") for h in range(2)]
        unpack_jit = jax.jit(_unpack_cpu, backend="cpu")

        bass2jax.install_neuronx_cc_hook()
        pn = nc.partition_id_tensor.name if nc.partition_id_tensor else None
        in_names, out_names, out_avals = [], [], []
        for alloc in nc.m.functions[0].allocations:
            if not isinstance(alloc, mybir.MemoryLocationSet):
                continue
            name = alloc.memorylocations[0].name
            if alloc.kind == "ExternalInput":
                if name != pn:
                    in_names.append(name)
            elif alloc.kind == "ExternalOutput":
                out_names.append(name)
                out_avals.append(jax.core.ShapedArray(
                    tuple(alloc.tensor_shape), mybir.dt.np(alloc.dtype)))
        assert in_names == ["x0", "x1"], in_names
        n_params, n_outs = len(in_names), len(out_avals)
        in_names_all = in_names + out_names + ([pn] if pn else [])
        donate = tuple(range(n_params, n_params + n_outs))

        def _body(*args):
            operands = list(args)
            if pn is not None:
                operands.append(bass2jax.partition_id_tensor())
            return tuple(bass2jax._bass_exec_p.bind(
                *operands, out_avals=tuple(out_avals),
                in_names=tuple(in_names_all), out_names=tuple(out_names),
                lowering_input_output_aliases=(),
                sim_require_finite=True, sim_require_nnan=True, nc=nc))

        devices = jax.devices()[:N_CORES]
        mesh = Mesh(np.asarray(devices), ("core",))
        spec = PartitionSpec("core")
        shd = NamedSharding(mesh, spec)
        sharded = jax.jit(
            shard_map(_body, mesh=mesh, in_specs=(spec,) * (n_params + n_outs),
                      out_specs=(spec,) * n_outs, check_rep=False),
            donate_argnums=donate, keep_unused=True)
        g_out = (N_CORES * out_avals[0].shape[0], *out_avals[0].shape[1:])
        zeros_jit = jax.jit(lambda: jnp.zeros(g_out, out_avals[0].dtype),
                            out_shardings=shd)

        # warm-compile + cross-check the fast path against the official run
        xg = np.zeros((N_CORES * BH, D_IN), np.int32)
        out = sharded(xg, xg, zeros_jit())
        w_fast = np.asarray(out[0])
        assert np.array_equal(w_fast, w_official), "fast path mismatch"
        pack_jits[0](np.zeros((4096, 64), np.float32))
        unpack_jit(np.zeros((4096, 1), np.int32))

        _STATE.update(dict(pack_jits=pack_jits, unpack_jit=unpack_jit,
                           sharded=sharded, zeros_jit=zeros_jit, shd=shd,
                           ready=True))
    except Exception as e:  # fall back to the plain spmd path per call
        _STATE["failed"] = repr(e)
        if "nc" not in _STATE:
            _STATE["nc"] = _build()


def _prepare():
    with _LOCK:
        _prepare_locked()


def _get_nc():
    _prepare()
    return _STATE["nc"]


_WARM = threading.Thread(target=_prepare, daemon=True)
_WARM.start()


def kernel(fp64_pulse: np.ndarray) -> np.ndarray:
    x = np.asarray(fp64_pulse)
    assert x.shape == (B, 64)
    _prepare()
    if "ready" in _STATE:
        try:
            S = _STATE
            zeros = S["zeros_jit"]()                     # async, on-device
            a0 = np.asarray(S["pack_jits"][0](x))
            d0 = jax.device_put(a0, S["shd"])            # async upload half 0
            a1 = np.asarray(S["pack_jits"][1](x))        # overlaps d0 upload
            out = S["sharded"](d0, a1, zeros)
            w = np.asarray(out[0])                       # (B, 1) int32
            return np.asarray(S["unpack_jit"](w))
        except Exception:
            pass  # transient failure: serve this call via the plain path
    # fallback: plain official path with numpy pack/unpack
    nc = _STATE["nc"]
    h0 = _pack_half_np(x, 0)
    h1 = _pack_half_np(x, 1)
    in_maps = [{"x0": h0[c * BH:(c + 1) * BH], "x1": h1[c * BH:(c + 1) * BH]}
               for c in range(N_CORES)]
    res = run_bass_kernel_spmd(nc, in_maps, core_ids=list(range(N_CORES)))
    w = np.concatenate([r["y"] for r in res.results], axis=0)
    return _unpack_output_np(w)


# revision 10
# speedup vs baseline: 2.6305x; 2.6305x over previous
"""FP64->FP32 bit-circuit converter for Trainium2 (8 NeuronCores), packed I/O.

The end-to-end cost of kernel() is dominated by host<->device transport over
the axon tunnel (~50-150 MB/s, high fixed latency), not device execution.
Strategy (pure data parallel over the batch, 131072 rows/core):

  host:   pack the (B, 64) {0,1}-float input into 2 int32 words per row
          (8 MB instead of 256 MB) with an XLA-CPU jit, in two halves so the
          first half's async upload overlaps packing of the second half (the
          two global arrays also double the number of transfer streams);
  device: run the whole conversion as ~35 fused int32 ALU ops per row on the
          vector engine (int32 bitwise is DVE-only; Pool-engine int ops
          measured ~30x slower), emitting the literal IEEE fp32 bit pattern
          as one int32 per row (4 MB back instead of 128 MB);
  host:   expand the words back into the (B, 32) float bit matrix.

Bit layout (MSB-first, matching the column order of the reference):
  hi = row bits 0..31  (bit0=sign at bit31, bits1..11=exp, bits12..31=mant0..19)
  lo = row bits 32..63 (mant bits 20..51)
Output word = sign<<31 | exp8<<23 | mant23 -- exactly the fp32 bit pattern,
whose MSB-first bit expansion equals the 32 output columns.

Half h of the input feeds supertile h: partition p of tensor x<h> holds core
rows [p*1024 + h*512, p*1024 + (h+1)*512), so the output row order matches.

The Bass kernel is compiled and first executed via
bass_utils.run_bass_kernel_spmd (during warm-up, which also cross-checks the
fast path against it); steady-state calls reuse one cached jit executor with
donated output buffers created on-device.  Warm-up starts in a background
thread at import so axon connection + neuronxcc compile overlap harness setup.
"""
import threading
import numpy as np
import jax
import jax.numpy as jnp
from jax.sharding import Mesh, PartitionSpec, NamedSharding
from jax.experimental.shard_map import shard_map

from concourse import bacc, bass2jax, mybir
from concourse.tile import TileContext
from concourse.bass_utils import run_bass_kernel_spmd

AOT = mybir.AluOpType
I32 = mybir.dt.int32

B = 1_048_576
N_CORES = 8
B_CORE = B // N_CORES          # 131072
P = 128                        # SBUF partitions
NI = B_CORE // P               # 1024 rows per partition
NF = NI // 2                   # 512 rows/partition per supertile
BH = B_CORE // 2               # 65536 rows per half per core
D_IN = 2                       # packed words per row


def _build():
    nc = bacc.Bacc("TRN2")
    x0 = nc.dram_tensor("x0", [BH, D_IN], I32, kind="ExternalInput")
    x1 = nc.dram_tensor("x1", [BH, D_IN], I32, kind="ExternalInput")
    y = nc.dram_tensor("y", [B_CORE, 1], I32, kind="ExternalOutput")
    y_r = y.ap().rearrange("(p n) d -> p (n d)", p=P)
    xrs = [t.ap().rearrange("(p n) d -> p (n d)", p=P) for t in (x0, x1)]

    with TileContext(nc) as tc:
        with (
            tc.tile_pool(name="io", bufs=2) as io,
            tc.tile_pool(name="sc", bufs=2) as sc,
        ):
            for st in range(2):
                off = st * NF
                xin = io.tile([P, NF * D_IN], I32, tag="xin", name="xin")
                nc.sync.dma_start(xin[:, :], xrs[st][:, :])
                xv = xin[:, :].rearrange("p (n d) -> p n d", d=D_IN)
                hi = xv[:, :, 0]
                lo = xv[:, :, 1]

                def T(tag):
                    t = sc.tile([P, NF], I32, tag=tag, name=tag)
                    return t[:, :]

                V = nc.vector
                # field extraction
                E = T("E")          # 11-bit biased fp64 exponent
                V.tensor_scalar(E, hi, 20, 0x7FF,
                                AOT.logical_shift_right, AOT.bitwise_and)
                Mhi = T("Mhi")
                V.tensor_scalar(Mhi, hi, 0xFFFFF, 3,
                                AOT.bitwise_and, AOT.logical_shift_left)
                Mlo = T("Mlo")
                V.tensor_scalar(Mlo, lo, 29, 7,
                                AOT.logical_shift_right, AOT.bitwise_and)
                M23 = T("M23")      # top 23 mantissa bits as an int
                V.tensor_tensor(M23, Mhi, Mlo, AOT.bitwise_or)
                R = T("R")          # round bit (mant bit 23)
                V.tensor_scalar(R, lo, 28, 1,
                                AOT.logical_shift_right, AOT.bitwise_and)
                sval = T("sval")    # sticky field (mant bits 24..51)
                V.tensor_scalar(sval, lo, 0x0FFFFFFF, None, AOT.bitwise_and)
                S = T("S")
                V.tensor_scalar(S, sval, 1, None, AOT.min)
                # round-to-nearest-even: round_up = R & (S | lsb)
                L = T("L")
                V.tensor_scalar(L, M23, 1, None, AOT.bitwise_and)
                SL = T("SL")
                V.tensor_tensor(SL, S, L, AOT.bitwise_or)
                ru = T("ru")
                V.tensor_tensor(ru, R, SL, AOT.bitwise_and)
                Mr = T("Mr")
                V.tensor_tensor(Mr, M23, ru, AOT.add)
                c_m = T("c_m")      # mantissa carry into the exponent
                V.tensor_scalar(c_m, Mr, 23, None, AOT.logical_shift_right)
                mant_f = T("mant_f")
                V.tensor_scalar(mant_f, Mr, 0x7FFFFF, None, AOT.bitwise_and)
                # rebias: newE = (E - 896) + c_m
                newE = T("newE")
                V.scalar_tensor_tensor(newE, E, -896, c_m, AOT.add, AOT.add)
                nsh = T("nsh")
                V.tensor_scalar(nsh, newE, 23, None, AOT.logical_shift_left)
                body = T("body")
                V.tensor_tensor(body, nsh, mant_f, AOT.bitwise_or)
                # specials
                over = T("over")
                V.tensor_scalar(over, E, 1151, None, AOT.is_ge)
                under = T("under")
                V.tensor_scalar(under, E, 897, None, AOT.is_lt)
                lml = T("lml")      # mant bits 23..51
                V.tensor_scalar(lml, lo, 0x1FFFFFFF, None, AOT.bitwise_and)
                manyv = T("manyv")
                V.tensor_tensor(manyv, M23, lml, AOT.bitwise_or)
                eq2047 = T("eq2047")
                V.tensor_scalar(eq2047, E, 2047, None, AOT.is_equal)
                many = T("many")
                V.tensor_scalar(many, manyv, 1, None, AOT.min)
                nan = T("nan")
                V.tensor_tensor(nan, eq2047, many, AOT.bitwise_and)
                om = T("om")
                V.tensor_scalar(om, over, 1, None, AOT.subtract)
                um = T("um")
                V.tensor_scalar(um, under, 1, None, AOT.subtract)
                nm = T("nm")
                V.tensor_scalar(nm, nan, 1, None, AOT.subtract)
                sb = T("sb")
                V.tensor_scalar(sb, hi, 31, 31,
                                AOT.logical_shift_right, AOT.logical_shift_left)
                # body1 = over ? 0x7F800000 : body   (xor/and with NOT-mask)
                x1t = T("x1t")
                V.tensor_scalar(x1t, body, 0x7F800000, None, AOT.bitwise_xor)
                x2t = T("x2t")
                V.tensor_tensor(x2t, x1t, om, AOT.bitwise_and)
                body1 = T("body1")
                V.tensor_scalar(body1, x2t, 0x7F800000, None, AOT.bitwise_xor)
                # body2 = under ? 0 : body1
                body2 = T("body2")
                V.tensor_tensor(body2, body1, um, AOT.bitwise_and)
                # body3 = nan ? 0x7FC00000 : body2
                x3t = T("x3t")
                V.tensor_scalar(x3t, body2, 0x7FC00000, None, AOT.bitwise_xor)
                x4t = T("x4t")
                V.tensor_tensor(x4t, x3t, nm, AOT.bitwise_and)
                body3 = T("body3")
                V.tensor_scalar(body3, x4t, 0x7FC00000, None, AOT.bitwise_xor)
                yt = io.tile([P, NF], I32, tag="yt", name="yt")
                V.tensor_tensor(yt[:, :], body3, sb, AOT.bitwise_or)
                nc.sync.dma_start(y_r[:, off:off + NF], yt[:, :])
    nc.compile()
    return nc


# ---------------- host-side pack / unpack (XLA CPU) ----------------
_PACK_W = (np.uint32(1) << np.arange(31, -1, -1, dtype=np.uint32)).astype(np.int32)


def _pack_half_cpu(xf, h):
    # partition p of half h holds core rows [p*NI + h*NF, p*NI + (h+1)*NF)
    xr = xf.reshape(N_CORES, P, 2, NF, 64)[:, :, h]
    xi = xr.reshape(-1, D_IN, 32).astype(jnp.int32)
    return (xi * _PACK_W[None, None, :]).sum(axis=-1, dtype=jnp.int32)


def _unpack_cpu(w):
    sh = jnp.arange(31, -1, -1, dtype=jnp.int32)
    bits = jnp.right_shift(w.reshape(-1, 1).view(jnp.uint32),
                           sh.view(jnp.uint32)[None, :]) & jnp.uint32(1)
    return bits.astype(jnp.float32)


def _pack_half_np(x: np.ndarray, h: int) -> np.ndarray:
    xr = np.ascontiguousarray(
        x.reshape(N_CORES, P, 2, NF, 64)[:, :, h]).reshape(-1, 64)
    xp = np.packbits(xr != 0, axis=-1)
    return xp.view(np.dtype(">u4")).astype(np.uint32).view(np.int32)


def _unpack_output_np(w: np.ndarray) -> np.ndarray:
    wbe = w.view(np.uint32).astype(np.dtype(">u4"))
    bits = np.unpackbits(wbe.view(np.uint8).reshape(-1, 4), axis=-1)
    return bits.astype(np.float32)


# ---------------- cached executor ----------------
_STATE: dict = {}
_LOCK = threading.Lock()


def _prepare_locked():
    if "ready" in _STATE or "failed" in _STATE:
        return
    try:
        nc = _build()
        _STATE["nc"] = nc

        # official path first: compile + run the Bass kernel via
        # run_bass_kernel_spmd (dummy input); also warms devices + NEFF.
        dummy = np.zeros((BH, D_IN), np.int32)
        in_maps = [{"x0": dummy, "x1": dummy} for _ in range(N_CORES)]
        res = run_bass_kernel_spmd(nc, in_maps, core_ids=list(range(N_CORES)))
        w_official = np.concatenate([r["y"] for r in res.results], axis=0)

        pack_jits = [jax.jit(lambda x, hh=h: _pack_half_cpu(x, hh),
                             backend="cpu") for h in range(2)]
        unpack_jit = jax.jit(_unpack_cpu, backend="cpu")

        bass2jax.install_neuronx_cc_hook()
        pn = nc.partition_id_tensor.name if nc.partition_id_tensor else None
        in_names, out_names, out_avals = [], [], []
        for alloc in nc.m.functions[0].allocations:
            if not isinstance(alloc, mybir.MemoryLocationSet):
                continue
            name = alloc.memorylocations[0].name
            if alloc.kind == "ExternalInput":
                if name != pn:
                    in_names.append(name)
            elif alloc.kind == "ExternalOutput":
                out_names.append(name)
                out_avals.append(jax.core.ShapedArray(
                    tuple(alloc.tensor_shape), mybir.dt.np(alloc.dtype)))
        assert in_names == ["x0", "x1"], in_names
        n_params, n_outs = len(in_names), len(out_avals)
        in_names_all = in_names + out_names + ([pn] if pn else [])
        donate = tuple(range(n_params, n_params + n_outs))

        def _body(*args):
            operands = list(args)
            if pn is not None:
                operands.append(bass2jax.partition_id_tensor())
            return tuple(bass2jax._bass_exec_p.bind(
                *operands, out_avals=tuple(out_avals),
                in_names=tuple(in_names_all), out_names=tuple(out_names),
                lowering_input_output_aliases=(),
                sim_require_finite=True, sim_require_nnan=True, nc=nc))

        devices = jax.devices()[:N_CORES]
        mesh = Mesh(np.asarray(devices), ("core",))
        spec = PartitionSpec("core")
        shd = NamedSharding(mesh, spec)
        sharded = jax.jit(
            shard_map(_body, mesh=mesh, in_specs=(spec,) * (n_params + n_outs),
                      out_specs=(spec,) * n_outs, check_rep=False),
            donate_argnums=donate, keep_unused=True)
        g_out = (N_CORES * out_avals[0].shape[0], *out_avals[0].shape[1:])
        zeros_jit = jax.jit(lambda: jnp.zeros(g_out, out_avals[0].dtype),
                            out_shardings=shd)

        # warm-compile + cross-check the fast path against the official run
        xg = np.zeros((N_CORES * BH, D_IN), np.int32)
        out = sharded(xg, xg, zeros_jit())
        w_fast = np.asarray(out[0])
        assert np.array_equal(w_fast, w_official), "fast path mismatch"
        xf0 = np.zeros((B, 64), np.float32)
        pack_jits[0](xf0)
        pack_jits[1](xf0)
        unpack_jit(np.zeros((B, 1), np.int32))

        _STATE.update(dict(pack_jits=pack_jits, unpack_jit=unpack_jit,
                           sharded=sharded, zeros_jit=zeros_jit, shd=shd,
                           ready=True))
    except Exception as e:  # fall back to the plain spmd path per call
        _STATE["failed"] = repr(e)
        if "nc" not in _STATE:
            _STATE["nc"] = _build()


def _prepare():
    with _LOCK:
        _prepare_locked()


def _get_nc():
    _prepare()
    return _STATE["nc"]


_WARM = threading.Thread(target=_prepare, daemon=True)
_WARM.start()


def kernel(fp64_pulse: np.ndarray) -> np.ndarray:
    x = np.asarray(fp64_pulse)
    assert x.shape == (B, 64)
    _prepare()
    if "ready" in _STATE:
        try:
            S = _STATE
            zeros = S["zeros_jit"]()                     # async, on-device
            a0 = np.asarray(S["pack_jits"][0](x))
            d0 = jax.device_put(a0, S["shd"])            # async upload half 0
            a1 = np.asarray(S["pack_jits"][1](x))        # overlaps d0 upload
            out = S["sharded"](d0, a1, zeros)
            w = np.asarray(out[0])                       # (B, 1) int32
            return np.asarray(S["unpack_jit"](w))
        except Exception:
            pass  # transient failure: serve this call via the plain path
    # fallback: plain official path with numpy pack/unpack
    nc = _STATE["nc"]
    h0 = _pack_half_np(x, 0)
    h1 = _pack_half_np(x, 1)
    in_maps = [{"x0": h0[c * BH:(c + 1) * BH], "x1": h1[c * BH:(c + 1) * BH]}
               for c in range(N_CORES)]
    res = run_bass_kernel_spmd(nc, in_maps, core_ids=list(range(N_CORES)))
    w = np.concatenate([r["y"] for r in res.results], axis=0)
    return _unpack_output_np(w)
